# revision 12
# baseline (speedup 1.0000x reference)
"""Trainium2 Bass kernel for nn_DecoderLayer (GNN message passing layer).

Data-parallel over the node axis N=4096 across 8 NeuronCores (512
nodes/core). Heavy compute runs feature-major ([C, rows] in SBUF) so every
matmul streams wide moving operands at full fp32r rate with constant
stationary weights. Edge features are pre-transposed/interleaved on the
host so device DMAs are fully contiguous and run at the HBM roofline.

Deep software pipeline over super-blocks of 32 nodes (1536 edge rows); in
period t the engines work on different super-blocks so every cross-engine
dependency has about a full period of slack:
  DMA : edges(t+2)
  PE  : m1(t) (3 edge chunks + stride-0-broadcast node chunk),
        m3(t-2), m2(t-1), + dense-phase matmuls
  ACT : gelu1(t) (eager per 384-slice), gelu2(t-1)
  DVE : attn-mult(t-2), k=48 aggregation(t-2)
  GPS : attention row broadcast
The small dense part (residual + LN + MLP + LN + mask) is processed in
4 chunks of 128 nodes, each overlapped with the main loop as soon as its
aggregates are ready.
"""

import numpy as np
from contextlib import ExitStack

import concourse.bacc as bacc
import concourse.tile as tile
from concourse import mybir
from concourse._compat import with_exitstack
from concourse.bass_utils import run_bass_kernel_spmd

F32 = mybir.dt.float32
F32R = mybir.dt.float32r
GELU = mybir.ActivationFunctionType.Gelu
IDENT = mybir.ActivationFunctionType.Identity
SQRT = mybir.ActivationFunctionType.Sqrt
SQUARE = mybir.ActivationFunctionType.Square
ADD = mybir.AluOpType.add
SUB = mybir.AluOpType.subtract
MULT = mybir.AluOpType.mult
AXX = mybir.AxisListType.X

# Problem constants
N, K, C, ECTX, HID = 4096, 48, 128, 384, 512
NCORES = 8
NN = N // NCORES            # nodes per core = 512
R = NN * K                  # edge rows per core = 24576
SBN = 32                    # nodes per super-block
SBR = SBN * K               # rows per super-block = 1536
NSB = NN // SBN             # super-blocks per core = 16
EPS = 1e-5
SCALE = 30.0


@with_exitstack
def _decoder_kernel(ctx: ExitStack, tc: tile.TileContext, aps: dict):
    nc = tc.nc

    consts = ctx.enter_context(tc.tile_pool(name="consts", bufs=1))
    slps = ctx.enter_context(tc.tile_pool(name="slps", bufs=2, space="PSUM"))
    ps1p = ctx.enter_context(tc.tile_pool(name="ps1p", bufs=1, space="PSUM"))
    ps3p = ctx.enter_context(tc.tile_pool(name="ps3p", bufs=1, space="PSUM"))
    epool = ctx.enter_context(tc.tile_pool(name="epool", bufs=3))
    a1pool = ctx.enter_context(tc.tile_pool(name="a1pool", bufs=3))
    abpool = ctx.enter_context(tc.tile_pool(name="abpool", bufs=2))
    hpool = ctx.enter_context(tc.tile_pool(name="hpool", bufs=2))
    dpool = ctx.enter_context(tc.tile_pool(name="dpool", bufs=4))
    small = ctx.enter_context(tc.tile_pool(name="small", bufs=4))

    edges = aps["edges"]
    attn = aps["attn"]
    st = {}

    def dma_edges(t):
        eT = epool.tile([128, 3 * SBR], F32R, tag="eT")
        nc.sync.dma_start(eT[:], edges[:, t * 3 * SBR:(t + 1) * 3 * SBR])
        st.setdefault(t, {})["eT"] = eT

    def dma_attn(t):
        at1 = a1pool.tile([1, SBR], F32R, tag="at1")
        nc.sync.dma_start(at1[:], attn[:, t * SBR:(t + 1) * SBR])
        st.setdefault(t, {})["at1"] = at1

    # start streaming edges before anything else
    dma_edges(0)
    dma_attn(0)
    dma_edges(1)

    def load_const(name, shape, dtype):
        t = consts.tile(shape, dtype, tag=name)
        nc.sync.dma_start(t[:], aps[name][:])
        return t

    w1e = load_const("w1e", [128, 3, 128], F32R)
    w1n = load_const("w1n", [128, 128], F32R)
    w2 = load_const("w2", [128, 128], F32R)
    w3 = load_const("w3", [128, 128], F32R)
    wd1 = load_const("wd1", [128, HID], F32R)
    wd2 = load_const("wd2", [128, 4, 128], F32R)
    b1c = load_const("b1c", [128, 1], F32)
    b2c = load_const("b2c", [128, 1], F32)
    b3r = load_const("b3r", [1, 128], F32R)
    bd1 = load_const("bd1", [128, 4], F32)
    bd2 = load_const("bd2", [128, 1], F32)
    g1r = load_const("g1r", [128, 128], F32)
    be1r = load_const("be1r", [128, 128], F32)
    g2r = load_const("g2r", [128, 128], F32)
    be2r = load_const("be2r", [128, 128], F32)
    ident = load_const("ident", [128, 128], F32)
    node_t = load_const("node_t", [128, NN], F32)
    sum_a = load_const("sum_a", [1, NN], F32R)
    mask_t = load_const("mask_t", [128, 4], F32)

    # rounded copy of node features for fp32r matmul input
    node_r = consts.tile([128, NN], F32R, tag="node_r")
    nc.vector.tensor_copy(node_r[:], node_t[:])

    eps_c = consts.tile([128, 1], F32, tag="eps_c")
    nc.vector.memset(eps_c[:], float(EPS))
    warm = consts.tile([128, 1], F32, tag="warm")
    nc.scalar.activation(warm[:], eps_c[:], SQRT)

    agg = consts.tile([128, NN], F32, tag="agg")
    z1n = consts.tile([128, NN], F32, tag="z1n")
    psz = slps.tile([128, NN], F32, tag="sl")
    nc.tensor.matmul(psz[:], w1n[:], node_r[:], start=True, stop=True)
    nc.scalar.copy(z1n[:], psz[:])

    def make_atb(t):
        atb = abpool.tile([128, SBR], F32R, tag="atb")
        nc.gpsimd.partition_broadcast(atb[:], st[t]["at1"][:])
        st[t]["atb"] = atb

    def stage_mult(t):
        # h2a = h2 * attn  (DVE; inputs ready a full period ago)
        s_ = st[t]
        h2a = hpool.tile([128, SBR], F32R, tag="h2a")
        nc.vector.tensor_tensor(h2a[:], s_["h2"][:], s_["atb"][:], op=MULT)
        s_["h2a"] = h2a

    def stageB(t):
        # m1 edge part: 3 contraction chunks x 3 col slices into one
        # 3-bank psum tile, then DVE adds the per-node z1n term and ACT
        # applies gelu in one wide pass each.
        s_ = st[t]
        eT = s_["eT"]
        ps1 = ps1p.tile([128, SBR], F32, tag="ps1")
        for c in range(3):
            for s in range(3):
                nc.tensor.matmul(
                    ps1[:, s * 512:(s + 1) * 512], w1e[:, c, :],
                    eT[:, c * SBR + s * 512: c * SBR + (s + 1) * 512],
                    start=(c == 0), stop=(c == 2))
        s_["ps1"] = ps1

    def stage_z1f(t):
        s_ = st[t]
        z1v = z1n[:, t * SBN:(t + 1) * SBN].unsqueeze(2) \
            .broadcast_to([128, SBN, K])
        z1f = hpool.tile([128, SBR], F32, tag="z1f")
        nc.vector.tensor_tensor(
            z1f[:].rearrange("p (n k) -> p n k", k=K),
            s_["ps1"][:].rearrange("p (n k) -> p n k", k=K),
            z1v, op=ADD)
        s_["z1f"] = z1f

    def stage_gelu1(t):
        s_ = st[t]
        h1 = hpool.tile([128, SBR], F32R, tag="h1")
        nc.scalar.activation(h1[:], s_["z1f"][:], GELU, bias=b1c[:, :])
        s_["h1"] = h1

    def stageC(t):
        s_ = st[t]
        h1 = s_["h1"]
        h2 = hpool.tile([128, SBR], F32R, tag="h2")
        for s in range(3):
            ps2 = slps.tile([128, 512], F32, tag="sl")
            nc.tensor.matmul(ps2[:], w2[:],
                             h1[:, s * 512:(s + 1) * 512],
                             start=True, stop=True)
            nc.scalar.activation(h2[:, s * 512:(s + 1) * 512], ps2[:],
                                 GELU, bias=b2c[:, :])
        s_["h2"] = h2

    def stage_m3(t):
        s_ = st[t]
        ps3 = ps3p.tile([128, SBR], F32, tag="ps3")
        for s in range(3):
            nc.tensor.matmul(
                ps3[:, s * 512:(s + 1) * 512], w3[:],
                s_["h2a"][:, s * 512:(s + 1) * 512], start=True, stop=True,
            )
        nc.vector.tensor_reduce(
            agg[:, t * SBN:(t + 1) * SBN],
            ps3[:].rearrange("p (n k) -> p n k", k=K),
            axis=AXX, op=ADD,
        )
        del st[t]

    def ln_chunk(x, g_rep, be_rep, out_t):
        """LayerNorm over the free dim (C=128) of a row-major [128,128] tile."""
        mu = small.tile([128, 1], F32, tag="mu")
        nc.vector.tensor_reduce(mu[:], x[:], axis=AXX, op=ADD)
        mu_s = small.tile([128, 1], F32, tag="mu_s")
        nc.vector.tensor_scalar_mul(mu_s[:], mu[:], 1.0 / 128.0)
        xc = dpool.tile([128, 128], F32, tag="xc")
        nc.vector.tensor_scalar(xc[:], x[:], mu_s[:, :], None, op0=SUB)
        sq = dpool.tile([128, 128], F32, tag="sq")
        vs = small.tile([128, 1], F32, tag="vs")
        nc.scalar.activation(sq[:], xc[:], SQUARE, accum_out=vs[:, :])
        sd = small.tile([128, 1], F32, tag="sd")
        nc.scalar.activation(sd[:], vs[:], SQRT, scale=1.0 / 128.0,
                             bias=eps_c[:, :])
        rstd = small.tile([128, 1], F32, tag="rstd")
        nc.vector.reciprocal(rstd[:], sd[:])
        xg = dpool.tile([128, 128], F32, tag="xg")
        nc.vector.scalar_tensor_tensor(xg[:], xc[:], rstd[:, :], g_rep[:],
                                       op0=MULT, op1=MULT)
        nc.vector.tensor_tensor(out_t[:], xg[:], be_rep[:], op=ADD)

    def ln_chunk_g(x, g_rep, be_rep, out_t):
        """Generator version of ln_chunk (yields between ops)."""
        mu = small.tile([128, 1], F32, tag="mu")
        nc.vector.tensor_reduce(mu[:], x[:], axis=AXX, op=ADD)
        mu_s = small.tile([128, 1], F32, tag="mu_s")
        nc.vector.tensor_scalar_mul(mu_s[:], mu[:], 1.0 / 128.0)
        yield
        xc = dpool.tile([128, 128], F32, tag="xc")
        nc.vector.tensor_scalar(xc[:], x[:], mu_s[:, :], None, op0=SUB)
        yield
        sq = dpool.tile([128, 128], F32, tag="sq")
        vs = small.tile([128, 1], F32, tag="vs")
        nc.scalar.activation(sq[:], xc[:], SQUARE, accum_out=vs[:, :])
        yield
        sd = small.tile([128, 1], F32, tag="sd")
        nc.scalar.activation(sd[:], vs[:], SQRT, scale=1.0 / 128.0,
                             bias=eps_c[:, :])
        rstd = small.tile([128, 1], F32, tag="rstd")
        nc.vector.reciprocal(rstd[:], sd[:])
        yield
        xg = dpool.tile([128, 128], F32, tag="xg")
        nc.vector.scalar_tensor_tensor(xg[:], xc[:], rstd[:, :], g_rep[:],
                                       op0=MULT, op1=MULT)
        nc.vector.tensor_tensor(out_t[:], xg[:], be_rep[:], op=ADD)
        yield

    def dense_chunk(ch):
        """Residual + LN1 + dense MLP + LN2 + mask for nodes
        [ch*128, (ch+1)*128), then write the output chunk. Generator:
        yields between dependent ops so 4 chunks interleave breadth-first."""
        sl = slice(ch * 128, (ch + 1) * 128)
        # x = nodeT + agg + outer(b3, sumA)   (feature-major)
        psbx = slps.tile([128, 128], F32, tag="sl")
        nc.tensor.matmul(psbx[:], b3r[:], sum_a[:, sl], start=True, stop=True)
        xt1 = dpool.tile([128, 128], F32, tag="xt1")
        nc.vector.tensor_tensor(xt1[:], node_t[:, sl], agg[:, sl], op=ADD)
        yield
        xTb = dpool.tile([128, 128], F32, tag="xTb")
        nc.vector.tensor_tensor(xTb[:], xt1[:], psbx[:], op=ADD)
        yield
        # to row-major for LN1
        pst = slps.tile([128, 128], F32, tag="sl")
        nc.tensor.transpose(pst[:], xTb[:], ident[:])
        x_rm = dpool.tile([128, 128], F32, tag="x_rm")
        nc.scalar.copy(x_rm[:], pst[:])
        yield
        x1n = dpool.tile([128, 128], F32, tag="x1n")
        yield from ln_chunk_g(x_rm, g1r, be1r, x1n)
        # back to feature-major for the MLP
        pst2 = slps.tile([128, 128], F32, tag="sl")
        nc.tensor.transpose(pst2[:], x1n[:], ident[:])
        x1nT = dpool.tile([128, 128], F32R, tag="x1nT")
        nc.scalar.copy(x1nT[:], pst2[:])
        yield
        hds = []
        for j in range(4):
            psd = slps.tile([128, 128], F32, tag="sl")
            nc.tensor.matmul(psd[:], wd1[:, j * 128:(j + 1) * 128], x1nT[:],
                             start=True, stop=True)
            h = dpool.tile([128, 128], F32R, tag=f"hd{j}")
            nc.scalar.activation(h[:], psd[:], GELU, bias=bd1[:, j:j + 1])
            hds.append(h)
            yield
        psd2 = slps.tile([128, 128], F32, tag="sl")
        for j in range(4):
            nc.tensor.matmul(psd2[:], wd2[:, j, :], hds[j][:],
                             start=(j == 0), stop=(j == 3))
        dT = dpool.tile([128, 128], F32, tag="dT")
        nc.scalar.activation(dT[:], psd2[:], IDENT, bias=bd2[:, :])
        yield
        # residual in row-major + LN2 + mask
        pst3 = slps.tile([128, 128], F32, tag="sl")
        nc.tensor.transpose(pst3[:], dT[:], ident[:])
        x2 = dpool.tile([128, 128], F32, tag="x2")
        nc.vector.tensor_tensor(x2[:], x1n[:], pst3[:], op=ADD)
        yield
        x2n = dpool.tile([128, 128], F32, tag="x2n")
        yield from ln_chunk_g(x2, g2r, be2r, x2n)
        o_sb = dpool.tile([128, 128], F32, tag="o_sb")
        nc.vector.tensor_tensor(
            o_sb[:], x2n[:],
            mask_t[:, ch:ch + 1].broadcast_to([128, 128]), op=MULT)
        nc.sync.dma_start(aps["out"][sl, :], o_sb[:])

    # ---- pipelined emission ----
    for t in range(NSB + 2):
        if 0 <= t - 2:
            make_atb(t - 2)              # gpsimd
            stage_mult(t - 2)            # DVE (inputs a full period old)
        if t < NSB:
            stageB(t)                    # PE m1
            stage_z1f(t)                 # DVE node-term add
        if 0 <= t - 2:
            stage_m3(t - 2)              # PE m3 + DVE aggregate
        if t < NSB:
            stage_gelu1(t)               # ACT
        if t + 2 < NSB:
            dma_edges(t + 2)
        if 1 <= t - 1 < NSB:
            dma_attn(t - 1)
        if 0 <= t - 1 < NSB:
            stageC(t - 1)                # PE m2 + ACT gelu2

    # dense phase: 4 chunks of 128 nodes, interleaved breadth-first so the
    # per-chunk serial chains pipeline across engines
    gens = [dense_chunk(ch) for ch in range(4)]
    alive = list(gens)
    while alive:
        nxt = []
        for g in alive:
            try:
                next(g)
                nxt.append(g)
            except StopIteration:
                pass
        alive = nxt


_CACHE = {}


def _build_program():
    if "nc" in _CACHE:
        return _CACHE["nc"]
    nc = bacc.Bacc("TRN2", target_bir_lowering=False, debug=False)
    aps = {}

    def din(name, shape, dtype):
        aps[name] = nc.dram_tensor(name, shape, dtype, kind="ExternalInput").ap()

    din("edges", [128, NSB * 3 * SBR], F32R)
    din("attn", [1, R], F32R)
    din("node_t", [128, NN], F32)
    din("sum_a", [1, NN], F32R)
    din("mask_t", [128, 4], F32)
    din("w1e", [128, 3, 128], F32R)
    din("w1n", [128, 128], F32R)
    din("w2", [128, 128], F32R)
    din("w3", [128, 128], F32R)
    din("wd1", [128, HID], F32R)
    din("wd2", [128, 4, 128], F32R)
    din("b1c", [128, 1], F32)
    din("b2c", [128, 1], F32)
    din("b3r", [1, 128], F32R)
    din("bd1", [128, 4], F32)
    din("bd2", [128, 1], F32)
    din("g1r", [128, 128], F32)
    din("be1r", [128, 128], F32)
    din("g2r", [128, 128], F32)
    din("be2r", [128, 128], F32)
    din("ident", [128, 128], F32)
    aps["out"] = nc.dram_tensor("out", [NN, C], F32, kind="ExternalOutput").ap()

    with tile.TileContext(nc) as tc:
        _decoder_kernel(tc, aps)
    nc.compile()
    _CACHE["nc"] = nc
    return nc


def _prep_shared(W_m1, b_m1, W_m2, b_m2, W_m3, b_m3, g1, beta1,
                 W_d1, b_d1, W_d2, b_d2, g2, beta2):
    f = np.float32
    rep = lambda v: np.ascontiguousarray(np.tile(np.asarray(v, f)[None, :],
                                                 (128, 1)))
    return {
        "w1e": np.ascontiguousarray(
            np.asarray(W_m1, f)[:, C:].T.reshape(3, 128, 128)
            .transpose(1, 0, 2)),
        "w1n": np.ascontiguousarray(np.asarray(W_m1, f)[:, :C].T),
        "w2": np.ascontiguousarray(np.asarray(W_m2, f).T),
        "w3": np.ascontiguousarray((np.asarray(W_m3, f) / SCALE).T),
        "wd1": np.ascontiguousarray(np.asarray(W_d1, f).T),
        "wd2": np.ascontiguousarray(
            np.asarray(W_d2, f).T.reshape(4, 128, 128).transpose(1, 0, 2)),
        "b1c": np.ascontiguousarray(np.asarray(b_m1, f)[:, None]),
        "b2c": np.ascontiguousarray(np.asarray(b_m2, f)[:, None]),
        "b3r": np.ascontiguousarray(np.asarray(b_m3, f)[None, :]),
        "bd1": np.ascontiguousarray(np.asarray(b_d1, f).reshape(4, 128).T),
        "bd2": np.ascontiguousarray(np.asarray(b_d2, f)[:, None]),
        "g1r": rep(g1), "be1r": rep(beta1), "g2r": rep(g2), "be2r": rep(beta2),
        "ident": np.eye(128, dtype=f),
    }


def kernel(node_features, layer_edge_features, mask, attention_mask,
           W_m1, b_m1, W_m2, b_m2, W_m3, b_m3, g1, beta1,
           W_d1, b_d1, W_d2, b_d2, g2, beta2):
    f = np.float32
    node_features = np.asarray(node_features, f)
    layer_edge_features = np.asarray(layer_edge_features, f)
    mask = np.asarray(mask, f)
    attention_mask = np.asarray(attention_mask, f)

    shared = _prep_shared(W_m1, b_m1, W_m2, b_m2, W_m3, b_m3, g1, beta1,
                          W_d1, b_d1, W_d2, b_d2, g2, beta2)

    in_maps = []
    for ci in range(NCORES):
        lo, hi = ci * NN, (ci + 1) * NN
        e = layer_edge_features[lo:hi].reshape(R, ECTX).T  # [384, R]
        edges_il = np.ascontiguousarray(
            e.reshape(3, 128, NSB, SBR).transpose(1, 2, 0, 3)
            .reshape(128, NSB * 3 * SBR))
        am = attention_mask[lo:hi]
        m = {
            "edges": edges_il,
            "attn": np.ascontiguousarray(am.reshape(1, R)),
            "node_t": np.ascontiguousarray(node_features[lo:hi].T),
            "sum_a": np.ascontiguousarray(
                (am.sum(axis=1) / SCALE).reshape(1, NN).astype(f)),
            "mask_t": np.ascontiguousarray(mask[lo:hi].reshape(4, 128).T),
        }
        m.update(shared)
        in_maps.append(m)

    nc = _build_program()
    res = run_bass_kernel_spmd(nc, in_maps, core_ids=list(range(NCORES)))
    out = np.concatenate([res.results[i]["out"] for i in range(NCORES)], axis=0)
    return out.astype(np.float32)


# revision 13
# speedup vs baseline: 1.3201x; 1.3201x over previous
"""Trainium2 Bass kernel for nn_DecoderLayer (GNN message passing layer).

Data-parallel over the node axis N=4096 across 8 NeuronCores (512
nodes/core). Heavy compute runs feature-major ([C, rows] in SBUF) so every
matmul streams wide moving operands at full fp32r rate with constant
stationary weights. Edge features are pre-transposed/interleaved on the
host so device DMAs are fully contiguous and run at the HBM roofline.

Deep software pipeline over super-blocks of 32 nodes (1536 edge rows); in
period t the engines work on different super-blocks so every cross-engine
dependency has about a full period of slack:
  DMA : edges(t+2)
  PE  : m1(t) (3 edge chunks + stride-0-broadcast node chunk),
        m3(t-2), m2(t-1), + dense-phase matmuls
  ACT : gelu1(t) (eager per 384-slice), gelu2(t-1)
  DVE : attn-mult(t-2), k=48 aggregation(t-2)
  GPS : attention row broadcast
The small dense part (residual + LN + MLP + LN + mask) is processed in
4 chunks of 128 nodes, each overlapped with the main loop as soon as its
aggregates are ready.
"""

import numpy as np
from contextlib import ExitStack

import concourse.bacc as bacc
import concourse.tile as tile
from concourse import mybir
from concourse._compat import with_exitstack
from concourse.bass_utils import run_bass_kernel_spmd

F32 = mybir.dt.float32
F32R = mybir.dt.float32r
GELU = mybir.ActivationFunctionType.Gelu
IDENT = mybir.ActivationFunctionType.Identity
SQRT = mybir.ActivationFunctionType.Sqrt
SQUARE = mybir.ActivationFunctionType.Square
ADD = mybir.AluOpType.add
SUB = mybir.AluOpType.subtract
MULT = mybir.AluOpType.mult
AXX = mybir.AxisListType.X

# Problem constants
N, K, C, ECTX, HID = 4096, 48, 128, 384, 512
NCORES = 8
NN = N // NCORES            # nodes per core = 512
R = NN * K                  # edge rows per core = 24576
SBN = 32                    # nodes per super-block
SBR = SBN * K               # rows per super-block = 1536
NSB = NN // SBN             # super-blocks per core = 16
EPS = 1e-5
SCALE = 30.0


@with_exitstack
def _decoder_kernel(ctx: ExitStack, tc: tile.TileContext, aps: dict):
    nc = tc.nc

    consts = ctx.enter_context(tc.tile_pool(name="consts", bufs=1))
    slps = ctx.enter_context(tc.tile_pool(name="slps", bufs=5, space="PSUM"))
    ps3p = ctx.enter_context(tc.tile_pool(name="ps3p", bufs=1, space="PSUM"))
    epool = ctx.enter_context(tc.tile_pool(name="epool", bufs=3))
    a1pool = ctx.enter_context(tc.tile_pool(name="a1pool", bufs=3))
    abpool = ctx.enter_context(tc.tile_pool(name="abpool", bufs=2))
    hpool = ctx.enter_context(tc.tile_pool(name="hpool", bufs=2))
    dpool = ctx.enter_context(tc.tile_pool(name="dpool", bufs=4))
    small = ctx.enter_context(tc.tile_pool(name="small", bufs=4))

    edges = aps["edges"]
    attn = aps["attn"]
    st = {}

    def dma_edges(t):
        eT = epool.tile([128, 3 * SBR], F32R, tag="eT")
        nc.sync.dma_start(eT[:], edges[:, t * 3 * SBR:(t + 1) * 3 * SBR])
        st.setdefault(t, {})["eT"] = eT

    def dma_attn(t):
        at1 = a1pool.tile([1, SBR], F32R, tag="at1")
        nc.sync.dma_start(at1[:], attn[:, t * SBR:(t + 1) * SBR])
        st.setdefault(t, {})["at1"] = at1

    # start streaming edges before anything else
    dma_edges(0)
    dma_attn(0)
    dma_edges(1)

    def load_const(name, shape, dtype):
        t = consts.tile(shape, dtype, tag=name)
        nc.sync.dma_start(t[:], aps[name][:])
        return t

    w1e = load_const("w1e", [128, 3, 128], F32R)
    w1n = load_const("w1n", [128, 128], F32R)
    w2 = load_const("w2", [128, 128], F32R)
    w3 = load_const("w3", [128, 128], F32R)
    wd1 = load_const("wd1", [128, HID], F32R)
    wd2 = load_const("wd2", [128, 4, 128], F32R)
    b1c = load_const("b1c", [128, 1], F32)
    b2c = load_const("b2c", [128, 1], F32)
    b3r = load_const("b3r", [1, 128], F32R)
    bd1 = load_const("bd1", [128, 4], F32)
    bd2 = load_const("bd2", [128, 1], F32)
    g1r = load_const("g1r", [128, 128], F32)
    be1r = load_const("be1r", [128, 128], F32)
    g2r = load_const("g2r", [128, 128], F32)
    be2r = load_const("be2r", [128, 128], F32)
    ident = load_const("ident", [128, 128], F32)
    node_t = load_const("node_t", [128, NN], F32)
    sum_a = load_const("sum_a", [1, NN], F32R)
    mask_t = load_const("mask_t", [128, 4], F32)

    # rounded copy of node features for fp32r matmul input
    node_r = consts.tile([128, NN], F32R, tag="node_r")
    nc.vector.tensor_copy(node_r[:], node_t[:])

    eps_c = consts.tile([128, 1], F32, tag="eps_c")
    nc.vector.memset(eps_c[:], float(EPS))
    warm = consts.tile([128, 1], F32, tag="warm")
    nc.scalar.activation(warm[:], eps_c[:], SQRT)

    agg = consts.tile([128, NN], F32, tag="agg")

    def make_atb(t):
        atb = abpool.tile([128, SBR], F32R, tag="atb")
        nc.gpsimd.partition_broadcast(atb[:], st[t]["at1"][:])
        st[t]["atb"] = atb

    def stageB(t):
        # m1: 3 edge chunks + broadcast node chunk, 384-wide slices
        # (node-aligned: 8 nodes x 48 neighbors per slice)
        s_ = st[t]
        eT = s_["eT"]
        h1 = hpool.tile([128, SBR], F32R, tag="h1")
        for q in range(4):
            ps1 = slps.tile([128, 384], F32, tag="sl")
            for c in range(3):
                nc.tensor.matmul(
                    ps1[:], w1e[:, c, :],
                    eT[:, c * SBR + q * 384: c * SBR + (q + 1) * 384],
                    start=(c == 0), stop=False)
            nv = node_r[:, t * SBN + q * 8: t * SBN + (q + 1) * 8] \
                .unsqueeze(2).broadcast_to([128, 8, K])
            nc.tensor.matmul(ps1[:].rearrange("p (n k) -> p n k", k=K),
                             w1n[:], nv, start=False, stop=True)
            nc.scalar.activation(h1[:, q * 384:(q + 1) * 384], ps1[:],
                                 GELU, bias=b1c[:, :])
        s_["h1"] = h1

    def stageC(t):
        s_ = st[t]
        h1 = s_["h1"]
        h2 = hpool.tile([128, SBR], F32R, tag="h2")
        for s in range(3):
            ps2 = slps.tile([128, 512], F32, tag="sl")
            nc.tensor.matmul(ps2[:], w2[:],
                             h1[:, s * 512:(s + 1) * 512],
                             start=True, stop=True)
            nc.scalar.activation(h2[:, s * 512:(s + 1) * 512], ps2[:],
                                 GELU, bias=b2c[:, :])
        s_["h2"] = h2

    def stageD(t):
        s_ = st[t]
        h2a = hpool.tile([128, SBR], F32R, tag="h2a")
        nc.vector.tensor_tensor(h2a[:], s_["h2"][:], s_["atb"][:], op=MULT)
        ps3 = ps3p.tile([128, SBR], F32, tag="ps3")
        for s in range(3):
            nc.tensor.matmul(
                ps3[:, s * 512:(s + 1) * 512], w3[:],
                h2a[:, s * 512:(s + 1) * 512], start=True, stop=True,
            )
        nc.vector.tensor_reduce(
            agg[:, t * SBN:(t + 1) * SBN],
            ps3[:].rearrange("p (n k) -> p n k", k=K),
            axis=AXX, op=ADD,
        )
        del st[t]

    def ln_chunk(x, g_rep, be_rep, out_t):
        """LayerNorm over the free dim (C=128) of a row-major [128,128] tile."""
        mu = small.tile([128, 1], F32, tag="mu")
        nc.vector.tensor_reduce(mu[:], x[:], axis=AXX, op=ADD)
        mu_s = small.tile([128, 1], F32, tag="mu_s")
        nc.vector.tensor_scalar_mul(mu_s[:], mu[:], 1.0 / 128.0)
        xc = dpool.tile([128, 128], F32, tag="xc")
        nc.vector.tensor_scalar(xc[:], x[:], mu_s[:, :], None, op0=SUB)
        sq = dpool.tile([128, 128], F32, tag="sq")
        vs = small.tile([128, 1], F32, tag="vs")
        nc.scalar.activation(sq[:], xc[:], SQUARE, accum_out=vs[:, :])
        sd = small.tile([128, 1], F32, tag="sd")
        nc.scalar.activation(sd[:], vs[:], SQRT, scale=1.0 / 128.0,
                             bias=eps_c[:, :])
        rstd = small.tile([128, 1], F32, tag="rstd")
        nc.vector.reciprocal(rstd[:], sd[:])
        xg = dpool.tile([128, 128], F32, tag="xg")
        nc.vector.scalar_tensor_tensor(xg[:], xc[:], rstd[:, :], g_rep[:],
                                       op0=MULT, op1=MULT)
        nc.vector.tensor_tensor(out_t[:], xg[:], be_rep[:], op=ADD)

    def ln_chunk_g(x, g_rep, be_rep, out_t):
        """Generator version of ln_chunk (yields between ops)."""
        mu = small.tile([128, 1], F32, tag="mu")
        nc.vector.tensor_reduce(mu[:], x[:], axis=AXX, op=ADD)
        mu_s = small.tile([128, 1], F32, tag="mu_s")
        nc.vector.tensor_scalar_mul(mu_s[:], mu[:], 1.0 / 128.0)
        yield
        xc = dpool.tile([128, 128], F32, tag="xc")
        nc.vector.tensor_scalar(xc[:], x[:], mu_s[:, :], None, op0=SUB)
        yield
        sq = dpool.tile([128, 128], F32, tag="sq")
        vs = small.tile([128, 1], F32, tag="vs")
        nc.scalar.activation(sq[:], xc[:], SQUARE, accum_out=vs[:, :])
        yield
        sd = small.tile([128, 1], F32, tag="sd")
        nc.scalar.activation(sd[:], vs[:], SQRT, scale=1.0 / 128.0,
                             bias=eps_c[:, :])
        rstd = small.tile([128, 1], F32, tag="rstd")
        nc.vector.reciprocal(rstd[:], sd[:])
        yield
        xg = dpool.tile([128, 128], F32, tag="xg")
        nc.vector.scalar_tensor_tensor(xg[:], xc[:], rstd[:, :], g_rep[:],
                                       op0=MULT, op1=MULT)
        nc.vector.tensor_tensor(out_t[:], xg[:], be_rep[:], op=ADD)
        yield

    def dense_chunk(ch):
        """Residual + LN1 + dense MLP + LN2 + mask for nodes
        [ch*128, (ch+1)*128), then write the output chunk. Generator:
        yields between dependent ops so 4 chunks interleave breadth-first."""
        sl = slice(ch * 128, (ch + 1) * 128)
        # x = nodeT + agg + outer(b3, sumA)   (feature-major)
        psbx = slps.tile([128, 128], F32, tag="sl")
        nc.tensor.matmul(psbx[:], b3r[:], sum_a[:, sl], start=True, stop=True)
        xt1 = dpool.tile([128, 128], F32, tag="xt1")
        nc.vector.tensor_tensor(xt1[:], node_t[:, sl], agg[:, sl], op=ADD)
        yield
        xTb = dpool.tile([128, 128], F32, tag="xTb")
        nc.vector.tensor_tensor(xTb[:], xt1[:], psbx[:], op=ADD)
        yield
        # to row-major for LN1
        pst = slps.tile([128, 128], F32, tag="sl")
        nc.tensor.transpose(pst[:], xTb[:], ident[:])
        x_rm = dpool.tile([128, 128], F32, tag="x_rm")
        nc.scalar.copy(x_rm[:], pst[:])
        yield
        x1n = dpool.tile([128, 128], F32, tag="x1n")
        yield from ln_chunk_g(x_rm, g1r, be1r, x1n)
        # back to feature-major for the MLP
        pst2 = slps.tile([128, 128], F32, tag="sl")
        nc.tensor.transpose(pst2[:], x1n[:], ident[:])
        x1nT = dpool.tile([128, 128], F32R, tag="x1nT")
        nc.scalar.copy(x1nT[:], pst2[:])
        yield
        hds = []
        for j in range(4):
            psd = slps.tile([128, 128], F32, tag="sl")
            nc.tensor.matmul(psd[:], wd1[:, j * 128:(j + 1) * 128], x1nT[:],
                             start=True, stop=True)
            h = dpool.tile([128, 128], F32R, tag=f"hd{j}")
            nc.scalar.activation(h[:], psd[:], GELU, bias=bd1[:, j:j + 1])
            hds.append(h)
            yield
        psd2 = slps.tile([128, 128], F32, tag="sl")
        for j in range(4):
            nc.tensor.matmul(psd2[:], wd2[:, j, :], hds[j][:],
                             start=(j == 0), stop=(j == 3))
        dT = dpool.tile([128, 128], F32, tag="dT")
        nc.scalar.activation(dT[:], psd2[:], IDENT, bias=bd2[:, :])
        yield
        # residual in row-major + LN2 + mask
        pst3 = slps.tile([128, 128], F32, tag="sl")
        nc.tensor.transpose(pst3[:], dT[:], ident[:])
        x2 = dpool.tile([128, 128], F32, tag="x2")
        nc.vector.tensor_tensor(x2[:], x1n[:], pst3[:], op=ADD)
        yield
        x2n = dpool.tile([128, 128], F32, tag="x2n")
        yield from ln_chunk_g(x2, g2r, be2r, x2n)
        o_sb = dpool.tile([128, 128], F32, tag="o_sb")
        nc.vector.tensor_tensor(
            o_sb[:], x2n[:],
            mask_t[:, ch:ch + 1].broadcast_to([128, 128]), op=MULT)
        nc.sync.dma_start(aps["out"][sl, :], o_sb[:])

    # ---- pipelined emission ----
    for t in range(NSB + 2):
        if 0 <= t - 2:
            make_atb(t - 2)              # gpsimd, feeds mult(t-2)
        if t < NSB:
            stageB(t)                    # PE m1 + ACT gelu1
        if 0 <= t - 2:
            stageD(t - 2)                # DVE mult, PE m3, DVE aggregate
        if t + 2 < NSB:
            dma_edges(t + 2)
        if 1 <= t - 1 < NSB:
            dma_attn(t - 1)
        if 0 <= t - 1 < NSB:
            stageC(t - 1)                # PE m2 + ACT gelu2

    # dense phase: 4 chunks of 128 nodes, interleaved breadth-first so the
    # per-chunk serial chains pipeline across engines
    gens = [dense_chunk(ch) for ch in range(4)]
    alive = list(gens)
    while alive:
        nxt = []
        for g in alive:
            try:
                next(g)
                nxt.append(g)
            except StopIteration:
                pass
        alive = nxt


_CACHE = {}


def _build_program():
    if "nc" in _CACHE:
        return _CACHE["nc"]
    nc = bacc.Bacc("TRN2", target_bir_lowering=False, debug=False)
    aps = {}

    def din(name, shape, dtype):
        aps[name] = nc.dram_tensor(name, shape, dtype, kind="ExternalInput").ap()

    din("edges", [128, NSB * 3 * SBR], F32R)
    din("attn", [1, R], F32R)
    din("node_t", [128, NN], F32)
    din("sum_a", [1, NN], F32R)
    din("mask_t", [128, 4], F32)
    din("w1e", [128, 3, 128], F32R)
    din("w1n", [128, 128], F32R)
    din("w2", [128, 128], F32R)
    din("w3", [128, 128], F32R)
    din("wd1", [128, HID], F32R)
    din("wd2", [128, 4, 128], F32R)
    din("b1c", [128, 1], F32)
    din("b2c", [128, 1], F32)
    din("b3r", [1, 128], F32R)
    din("bd1", [128, 4], F32)
    din("bd2", [128, 1], F32)
    din("g1r", [128, 128], F32)
    din("be1r", [128, 128], F32)
    din("g2r", [128, 128], F32)
    din("be2r", [128, 128], F32)
    din("ident", [128, 128], F32)
    aps["out"] = nc.dram_tensor("out", [NN, C], F32, kind="ExternalOutput").ap()

    with tile.TileContext(nc) as tc:
        _decoder_kernel(tc, aps)
    nc.compile()
    _CACHE["nc"] = nc
    return nc


def _prep_shared(W_m1, b_m1, W_m2, b_m2, W_m3, b_m3, g1, beta1,
                 W_d1, b_d1, W_d2, b_d2, g2, beta2):
    f = np.float32
    rep = lambda v: np.ascontiguousarray(np.tile(np.asarray(v, f)[None, :],
                                                 (128, 1)))
    return {
        "w1e": np.ascontiguousarray(
            np.asarray(W_m1, f)[:, C:].T.reshape(3, 128, 128)
            .transpose(1, 0, 2)),
        "w1n": np.ascontiguousarray(np.asarray(W_m1, f)[:, :C].T),
        "w2": np.ascontiguousarray(np.asarray(W_m2, f).T),
        "w3": np.ascontiguousarray((np.asarray(W_m3, f) / SCALE).T),
        "wd1": np.ascontiguousarray(np.asarray(W_d1, f).T),
        "wd2": np.ascontiguousarray(
            np.asarray(W_d2, f).T.reshape(4, 128, 128).transpose(1, 0, 2)),
        "b1c": np.ascontiguousarray(np.asarray(b_m1, f)[:, None]),
        "b2c": np.ascontiguousarray(np.asarray(b_m2, f)[:, None]),
        "b3r": np.ascontiguousarray(np.asarray(b_m3, f)[None, :]),
        "bd1": np.ascontiguousarray(np.asarray(b_d1, f).reshape(4, 128).T),
        "bd2": np.ascontiguousarray(np.asarray(b_d2, f)[:, None]),
        "g1r": rep(g1), "be1r": rep(beta1), "g2r": rep(g2), "be2r": rep(beta2),
        "ident": np.eye(128, dtype=f),
    }


def kernel(node_features, layer_edge_features, mask, attention_mask,
           W_m1, b_m1, W_m2, b_m2, W_m3, b_m3, g1, beta1,
           W_d1, b_d1, W_d2, b_d2, g2, beta2):
    f = np.float32
    node_features = np.asarray(node_features, f)
    layer_edge_features = np.asarray(layer_edge_features, f)
    mask = np.asarray(mask, f)
    attention_mask = np.asarray(attention_mask, f)

    shared = _prep_shared(W_m1, b_m1, W_m2, b_m2, W_m3, b_m3, g1, beta1,
                          W_d1, b_d1, W_d2, b_d2, g2, beta2)

    in_maps = []
    for ci in range(NCORES):
        lo, hi = ci * NN, (ci + 1) * NN
        e = layer_edge_features[lo:hi].reshape(R, ECTX).T  # [384, R]
        edges_il = np.ascontiguousarray(
            e.reshape(3, 128, NSB, SBR).transpose(1, 2, 0, 3)
            .reshape(128, NSB * 3 * SBR))
        am = attention_mask[lo:hi]
        m = {
            "edges": edges_il,
            "attn": np.ascontiguousarray(am.reshape(1, R)),
            "node_t": np.ascontiguousarray(node_features[lo:hi].T),
            "sum_a": np.ascontiguousarray(
                (am.sum(axis=1) / SCALE).reshape(1, NN).astype(f)),
            "mask_t": np.ascontiguousarray(mask[lo:hi].reshape(4, 128).T),
        }
        m.update(shared)
        in_maps.append(m)

    nc = _build_program()
    res = run_bass_kernel_spmd(nc, in_maps, core_ids=list(range(NCORES)))
    out = np.concatenate([res.results[i]["out"] for i in range(NCORES)], axis=0)
    return out.astype(np.float32)
